# revision 21
# baseline (speedup 1.0000x reference)
"""Multi-head attention (QKV proj + RoPE + SDPA + out proj) on 8 TRN2 NeuronCores.

Sharding: batch x head-group. Core c handles batch c//4 and heads
4*(c%4) .. 4*(c%4)+3 (4 of 16 heads, 256 of 1024 feature dims).

v2 design (all matmuls bf16 except the RoPE swap, N=1024 moving):
  - phase order: K-proj(all) -> Q-proj(qcp0) -> V-proj(all) -> attn(qcp0)
    -> Q-proj(qcp1) -> outproj(qcp0) -> attn(qcp1) -> outproj(qcp1).
    Q-proj(qcp1) fills the PE gap while qcp0's softmax denominators
    normalize; outproj(qcp0) runs while qcp1's attention would stall.
  - projections: x (bf16) resident per-dc tiles, weights bf16; PSUM f32;
    bias via ACT identity into f32 qsb; RoPE swap via pair-swap matmul
    (f32r, N=512 halves); rope mults on DVE in f32; single rounding into
    bf16 qrot/krot.
  - scores TRANSPOSED s[k, q] per head at N=1024; exp on ACT (scale 1/8
    folded, no max subtraction: |s|/8 < ~29, safe in f32/bf16 range).
    Optionally a subset of key-tiles' exps run on DVE as a Schraudolph
    int16/bf16 bit-trick (softmax normalization cancels its constant
    bias; only the +-3% mantissa sawtooth remains) to relieve the ACT
    engine, which is otherwise the attention-phase bottleneck.
  - attn@V with a ones-row per head (M=65): row 64 accumulates the
    softmax denominator free; normalize via reciprocal + partition
    broadcast (gpsimd for overlapped head-pairs, ACT+PE rank-1 for the
    critical tails); ynorm stored bf16.
  - out projection row-parallel, bf16 weights, bf16 partial out; host
    sums partials per batch in f32 and adds wo_b + wo_w @ wv_b (V bias
    commutes through softmax).
"""

import numpy as np
import ml_dtypes

import concourse.bass as bass
import concourse.mybir as mybir
import concourse.tile as tile
from concourse import bacc
import concourse.bass_utils as _bu
from concourse.bass_utils import run_bass_kernel_spmd

F32 = mybir.dt.float32
F32R = mybir.dt.float32r
BF16 = mybir.dt.bfloat16
I16 = mybir.dt.int16
AF = mybir.ActivationFunctionType
OP = mybir.AluOpType

B, S, D = 2, 2048, 1024
NH, HD = 16, 64
NCORES = 8
HPC = 4          # heads per core
DL = HPC * HD    # 256 local dims per core

TRACE = False
LAST_RESULTS = [None]
# key-tiles (of 16) whose exp runs on DVE via the corrected Schraudolph
# bit trick (i=1 head only, to balance ACT vs DVE load)
DVE_EXP_KTS = ()
STEADY_WARM = 1      # extra filler matmuls per kt in the attention loop
SCHRAUD_A = 0.125 * np.log2(np.e) * 128.0   # score -> int16 exponent scale
SCHRAUD_B = 16256.0                          # 127 * 128
# minimax-ish quadratic for 2^f/(1+f), f = (I & 127)/128
SC_A2, SC_A1, SC_A0 = 0.22697911, -0.21647824, 0.99383134


def _build_module():
    nc = bacc.Bacc("TRN2", target_bir_lowering=False, debug=False)

    xt_d = nc.dram_tensor("xt", [8, 128, S], BF16, kind="ExternalInput")
    wqt_d = nc.dram_tensor("wqt", [128, 8, DL], BF16, kind="ExternalInput")
    wkt_d = nc.dram_tensor("wkt", [128, 8, DL], BF16, kind="ExternalInput")
    wvt_d = nc.dram_tensor("wvt", [128, 8, DL], BF16, kind="ExternalInput")
    wot_d = nc.dram_tensor("wot", [128, 2, D], BF16, kind="ExternalInput")
    qb_d = nc.dram_tensor("qb2", [128, 2], F32, kind="ExternalInput")
    kb_d = nc.dram_tensor("kb2", [128, 2], F32, kind="ExternalInput")
    f0_d = nc.dram_tensor("f0", [128, S], F32, kind="ExternalInput")
    f1_d = nc.dram_tensor("f1", [128, S], F32, kind="ExternalInput")
    psw_d = nc.dram_tensor("pswap", [128, 128], F32R, kind="ExternalInput")
    o164_d = nc.dram_tensor("ones164", [1, 64], F32R, kind="ExternalInput")
    out_d = nc.dram_tensor("partial", [16, 128, D], BF16, kind="ExternalOutput")

    def act_reciprocal(out, in_):
        # ACT-engine reciprocal via direct emission (measured 1.2e-5 max rel)
        eng = nc.scalar
        ins_ = [eng.lower_ap(in_),
                mybir.ImmediateValue(dtype=F32, value=0.0),
                mybir.ImmediateValue(dtype=F32, value=1.0),
                mybir.ImmediateValue(dtype=F32, value=0.0)]
        eng.add_instruction(mybir.InstActivation(
            name=nc.get_next_instruction_name(),
            func=mybir.ActivationFunctionType.Reciprocal,
            ins=ins_, outs=[eng.lower_ap(out)]))

    with tile.TileContext(nc) as tc:
        with (
            tc.tile_pool(name="wts", bufs=1) as wpool,
            tc.tile_pool(name="persist", bufs=1) as ppool,
        ):
            # ---- weights / constants / x (resident), DMA'd in use order ----
            wkt = wpool.tile([128, 8, DL], BF16, tag="wkt")
            nc.sync.dma_start(out=wkt[:], in_=wkt_d.ap())
            xt_sb = []
            for dc in range(8):
                t = wpool.tile([128, S], BF16, tag=f"xt{dc}")
                nc.sync.dma_start(out=t[:], in_=xt_d.ap()[dc])
                xt_sb.append(t)
            kb = wpool.tile([128, 2], F32, tag="kb")
            nc.sync.dma_start(out=kb[:], in_=kb_d.ap())
            f0 = wpool.tile([128, S], F32, tag="f0")
            nc.sync.dma_start(out=f0[:], in_=f0_d.ap())
            f1 = wpool.tile([128, S], F32, tag="f1")
            nc.sync.dma_start(out=f1[:], in_=f1_d.ap())
            psw = wpool.tile([128, 128], F32R, tag="pswap")
            nc.sync.dma_start(out=psw[:], in_=psw_d.ap())
            wqt = wpool.tile([128, 8, DL], BF16, tag="wqt")
            nc.sync.dma_start(out=wqt[:], in_=wqt_d.ap())
            qb = wpool.tile([128, 2], F32, tag="qb")
            nc.sync.dma_start(out=qb[:], in_=qb_d.ap())
            wvt = wpool.tile([128, 8, DL], BF16, tag="wvt")
            nc.sync.dma_start(out=wvt[:], in_=wvt_d.ap())
            wot = wpool.tile([128, 2, D], BF16, tag="wot")
            nc.sync.dma_start(out=wot[:], in_=wot_d.ap())
            o164 = wpool.tile([1, 64], F32R, tag="o164")
            nc.sync.dma_start(out=o164[:], in_=o164_d.ap())

            # Schraudolph additive constants (exact in f32)
            scb = scb2 = None
            if DVE_EXP_KTS:
                scb = wpool.tile([128, 1024], F32, tag="scb")
                nc.vector.memset(scb[:], SCHRAUD_B)
                scb2 = wpool.tile([128, 1024], F32, tag="scb2")
                nc.vector.memset(scb2[:], SCHRAUD_B - 64.0)

            # persistent activations
            qrot = [ppool.tile([128, S], BF16, tag=f"qrot{pt}", name=f"qrot{pt}")
                     for pt in range(2)]
            krot = [ppool.tile([128, S], BF16, tag=f"krot{pt}", name=f"krot{pt}")
                     for pt in range(2)]
            ynorm = [ppool.tile([128, S], BF16, tag=f"ynorm{pt}", name=f"ynorm{pt}")
                     for pt in range(2)]
            vsb = [ppool.tile([128, 4, 65], BF16, tag=f"v{kt}", name=f"vsb{kt}")
                   for kt in range(16)]

            # preload the ACT exp table during the DMA lead-in
            warmact = wpool.tile([1, 1], F32, tag="warmact")
            nc.vector.memset(warmact[:], 0.0)
            nc.scalar.activation(warmact[:], warmact[:], AF.Exp, scale=1.0)

            def emit_proj(wt, bvec, rot, qc, pt, pp, swp, tp):
                """Project + RoPE one [1024-token x 128-dim] block."""
                tsl = slice(qc * 1024, (qc + 1) * 1024)
                qp = pp.tile([128, 1024], F32, tag="qp")
                for qh in range(2):
                    hsl = slice(qc * 1024 + qh * 512, qc * 1024 + (qh + 1) * 512)
                    for dc in range(8):
                        nc.tensor.matmul(
                            qp[:, qh * 512:(qh + 1) * 512],
                            wt[:, dc, pt * 128:(pt + 1) * 128],
                            xt_sb[dc][:, hsl], start=(dc == 0), stop=(dc == 7))
                qsb = tp.tile([128, 1024], F32R, tag="qsb")
                nc.scalar.activation(qsb[:], qp[:], AF.Identity,
                                     bias=bvec[:, pt:pt + 1], scale=1.0)
                sw = swp.tile([128, 1024], F32, tag="sw")
                for qh in range(2):
                    nc.tensor.matmul(sw[:, qh * 512:(qh + 1) * 512], psw[:],
                                     qsb[:, qh * 512:(qh + 1) * 512],
                                     start=True, stop=True)
                t0 = tp.tile([128, 1024], F32, tag="t0")
                nc.vector.tensor_tensor(t0[:], qsb[:], f0[:, tsl], OP.mult)
                t1 = tp.tile([128, 1024], F32, tag="t1")
                nc.vector.tensor_tensor(t1[:], sw[:], f1[:, tsl], OP.mult)
                nc.vector.tensor_tensor(rot[pt][:, tsl], t0[:], t1[:], OP.add)

            # ---- phase 1: K proj (all tokens), Q proj (qcp0) ----
            with (
                tc.tile_pool(name="ptmp", bufs=1) as tp,
                tc.tile_pool(name="pp", bufs=2, space="PSUM") as pp,
                tc.tile_pool(name="swp", bufs=2, space="PSUM") as swp,
            ):
                for qc in range(2):
                    for pt in range(2):
                        emit_proj(wkt, kb, krot, qc, pt, pp, swp, tp)
                for pt in range(2):
                    emit_proj(wqt, qb, qrot, 0, pt, pp, swp, tp)

            # ---- phase 2: V proj (token-major, all 16 key tiles) ----
            with tc.tile_pool(name="vps", bufs=2, space="PSUM") as vps:
                for kt in range(16):
                    vp = vps.tile([128, 256], F32, tag="vp")
                    for dc in range(8):
                        nc.tensor.matmul(
                            vp[:], xt_sb[dc][:, kt * 128:(kt + 1) * 128],
                            wvt[:, dc, :], start=(dc == 0), stop=(dc == 7))
                    nc.vector.tensor_copy(
                        vsb[kt][:, :, 0:64],
                        vp[:].rearrange("p (h c) -> p h c", c=64))
                    nc.vector.memset(vsb[kt][:, :, 64:65], 1.0)

            def warm_attn(aps, n):
                # dep-free fillers (krot only, stable) that keep the PE
                # clock-gate warm; target psum is overwritten right after
                for w in range(n):
                    wt_ = aps.tile([128, 1024], F32, tag="s0", name="warm")
                    nc.tensor.matmul(
                        wt_[:, 0:512], krot[0][0:64, 0:128],
                        krot[0][0:64, 0:512], start=True, stop=True)

            def emit_attention(qcp, epool, scpool, ypool, rpool, aps):
                q0 = qcp * 1024
                for hp in range(2):
                    pt = hp
                    warm_attn(aps, 3)
                    yps = [aps.tile([65, 1024], F32, tag=f"y{i}", name=f"yps{i}")
                           for i in range(2)]

                    def emit_scores_exp(kt, nwarm=0):
                        exs = []
                        for i in range(2):
                            if i == 0:
                                warm_attn(aps, nwarm)
                            sp = aps.tile([128, 1024], F32, tag=f"s{i}")
                            po = 64 * i
                            for qh in range(2):
                                nc.tensor.matmul(
                                    sp[:, qh * 512:(qh + 1) * 512],
                                    krot[pt][po:po + 64, kt * 128:(kt + 1) * 128],
                                    qrot[pt][po:po + 64,
                                             q0 + qh * 512:q0 + (qh + 1) * 512],
                                    start=True, stop=True)
                            if kt in DVE_EXP_KTS and i == 1:
                                # two-point Schraudolph exp on DVE:
                                # w0 = bitcast(round(A*s + B)) as bf16,
                                # w1 = bitcast(round(A*s + B - 64)); the
                                # half-period offset cancels most of the
                                # linear-mantissa sawtooth:
                                # w = w0 + sqrt(2)*w1 (constant factor
                                # cancels in softmax normalization)
                                exi = scpool.tile([128, 1024], I16, tag=f"ei{i}")
                                nc.vector.scalar_tensor_tensor(
                                    exi[:], sp[:], float(SCHRAUD_A), scb[:],
                                    OP.mult, OP.add)
                                exj = scpool.tile([128, 1024], I16, tag=f"ej{i}")
                                nc.vector.scalar_tensor_tensor(
                                    exj[:], sp[:], float(SCHRAUD_A), scb2[:],
                                    OP.mult, OP.add)
                                exf = epool.tile([128, 1024], BF16, tag=f"ex{i}")
                                nc.vector.scalar_tensor_tensor(
                                    exf[:], exj[:].bitcast(BF16), 1.41421356,
                                    exi[:].bitcast(BF16), OP.mult, OP.add)
                                exs.append(exf)
                            else:
                                ext = epool.tile([128, 1024], BF16, tag=f"e{i}")
                                nc.scalar.activation(ext[:], sp[:], AF.Exp,
                                                     scale=0.125)
                                exs.append(ext)
                        return exs

                    def emit_attnv(kt, exs):
                        for i in range(2):
                            h = 2 * hp + i
                            for qh in range(2):
                                nc.tensor.matmul(
                                    yps[i][:, qh * 512:(qh + 1) * 512],
                                    vsb[kt][:, h, :],
                                    exs[i][:, qh * 512:(qh + 1) * 512],
                                    start=(kt == 0), stop=(kt == 15))

                    prev = emit_scores_exp(0, 3)
                    for kt in range(1, 16):
                        exs = emit_scores_exp(kt, 2 if kt < 3 else STEADY_WARM)
                        emit_attnv(kt - 1, prev)
                        prev = exs
                    emit_attnv(15, prev)
                    if hp == 1:
                        # keep PE warm through the normalize tail
                        warm_attn(aps, 10)

                    for i in range(2):
                        h = 2 * hp + i
                        po = 64 * i
                        ysb = ypool.tile([65, 1024], F32, tag=f"ysb{i}")
                        nc.vector.tensor_copy(ysb[:], yps[i][:])
                        if hp == 1:
                            # critical tail: fast ACT reciprocal + rank-1 PE
                            # broadcast
                            rec = rpool.tile([1, 1024], F32R, tag=f"rec{i}")
                            act_reciprocal(rec[:], ysb[64:65, :])
                            nb = aps.tile([64, 1024], F32, tag=f"y{i}")
                            for qh in range(2):
                                nc.tensor.matmul(
                                    nb[:, qh * 512:(qh + 1) * 512], o164[:],
                                    rec[:, qh * 512:(qh + 1) * 512],
                                    start=True, stop=True)
                            nc.vector.tensor_tensor(
                                ynorm[pt][po:po + 64, q0:q0 + 1024],
                                ysb[0:64, :], nb[:], OP.mult)
                        else:
                            rraw = rpool.tile([1, 1024], F32, tag=f"rr{i}")
                            nc.vector.reciprocal(rraw[:], ysb[64:65, :])
                            rb = rpool.tile([64, 1024], F32, tag=f"rb{i}")
                            nc.gpsimd.partition_broadcast(
                                rb[:], rraw[:], channels=64)
                            nc.vector.tensor_tensor(
                                ynorm[pt][po:po + 64, q0:q0 + 1024],
                                ysb[0:64, :], rb[:], OP.mult)

            def emit_outproj(qcp, opp, obuf):
                for w in range(6):
                    wt_ = opp.tile([128, 1024], F32, tag="op0", name="warmo")
                    nc.tensor.matmul(
                        wt_[:, 0:512], krot[0][0:64, 0:128],
                        krot[0][0:64, 0:512], start=True, stop=True)
                for tt in range(8):
                    tg = qcp * 8 + tt
                    op = opp.tile([128, 1024], F32, tag=f"op{tt % 2}")
                    for oc in range(2):
                        for pt2 in range(2):
                            nc.tensor.matmul(
                                op[:, oc * 512:(oc + 1) * 512],
                                ynorm[pt2][:, tg * 128:(tg + 1) * 128],
                                wot[:, pt2, oc * 512:(oc + 1) * 512],
                                start=(pt2 == 0), stop=(pt2 == 1))
                    osb = obuf.tile([128, 1024], BF16, tag=f"osb{tt % 2}")
                    nc.vector.tensor_copy(osb[:], op[:])
                    nc.sync.dma_start(out=out_d.ap()[tg], in_=osb[:])

            with (
                tc.tile_pool(name="exp", bufs=2) as epool,
                tc.tile_pool(name="sct", bufs=1) as scpool,
                tc.tile_pool(name="ysb", bufs=1) as ypool,
                tc.tile_pool(name="rp", bufs=1) as rpool,
                tc.tile_pool(name="obuf", bufs=2) as obuf,
            ):
                with tc.tile_pool(name="aps0", bufs=1, space="PSUM") as aps:
                    emit_attention(0, epool, scpool, ypool, rpool, aps)
                with (
                    tc.tile_pool(name="ptmp1", bufs=1) as tp,
                    tc.tile_pool(name="pp1", bufs=2, space="PSUM") as pp,
                    tc.tile_pool(name="swp1", bufs=2, space="PSUM") as swp,
                ):
                    for pt in range(2):
                        emit_proj(wqt, qb, qrot, 1, pt, pp, swp, tp)
                with tc.tile_pool(name="opp0", bufs=2, space="PSUM") as opp:
                    emit_outproj(0, opp, obuf)
                with tc.tile_pool(name="aps1", bufs=1, space="PSUM") as aps:
                    emit_attention(1, epool, scpool, ypool, rpool, aps)
                with tc.tile_pool(name="opp1", bufs=2, space="PSUM") as opp:
                    emit_outproj(1, opp, obuf)

    nc.compile()
    return nc


_NC = None


def _get_module():
    global _NC
    if _NC is None:
        _NC = _build_module()
    return _NC


def _prep_in_maps(q, freqs_cis, wq_w, wq_b, wk_w, wk_b, wv_w, wv_b, wo_w, wo_b):
    BF = ml_dtypes.bfloat16
    # F0/F1 [128, S] (identical layout for every head pair on 128 partitions)
    i_of_p = (np.arange(128) % HD) // 2
    sign = np.where(np.arange(128) % 2 == 0, -1.0, 1.0).astype(np.float32)
    f0 = freqs_cis[:, i_of_p, 0].T.copy()                 # [128, S]
    f1 = (freqs_cis[:, i_of_p, 1].T * sign[:, None]).copy()
    pswap = np.zeros((128, 128), np.float32)
    idx = np.arange(128)
    pswap[idx ^ 1, idx] = 1.0
    ones164 = np.ones((1, 64), np.float32)

    def warr(w):  # [256, out-rows of W^T] -> [128, 8, 256] bf16
        return np.ascontiguousarray(
            w.T.reshape(8, 128, DL).transpose(1, 0, 2)).astype(BF)

    in_maps = []
    for c in range(NCORES):
        b, hg = c // 4, c % 4
        sl = slice(hg * DL, (hg + 1) * DL)
        in_maps.append({
            "xt": np.ascontiguousarray(q[b].T.reshape(8, 128, S)).astype(BF),
            "wqt": warr(wq_w[sl]),
            "wkt": warr(wk_w[sl]),
            "wvt": warr(wv_w[sl]),
            "wot": np.ascontiguousarray(
                wo_w[:, sl].T.reshape(2, 128, D).transpose(1, 0, 2)).astype(BF),
            "qb2": np.ascontiguousarray(wq_b[sl].reshape(2, 128).T),
            "kb2": np.ascontiguousarray(wk_b[sl].reshape(2, 128).T),
            "f0": f0,
            "f1": f1,
            "pswap": pswap,
            "ones164": ones164,
        })
    return in_maps


def kernel(q, freqs_cis, wq_w, wq_b, wk_w, wk_b, wv_w, wv_b, wo_w, wo_b):
    q = np.asarray(q, np.float32)
    freqs_cis = np.asarray(freqs_cis, np.float32)
    wq_w = np.asarray(wq_w, np.float32)
    wq_b = np.asarray(wq_b, np.float32)
    wk_w = np.asarray(wk_w, np.float32)
    wk_b = np.asarray(wk_b, np.float32)
    wv_w = np.asarray(wv_w, np.float32)
    wv_b = np.asarray(wv_b, np.float32)
    wo_w = np.asarray(wo_w, np.float32)
    wo_b = np.asarray(wo_b, np.float32)

    nc = _get_module()
    in_maps = _prep_in_maps(q, freqs_cis, wq_w, wq_b, wk_w, wk_b,
                            wv_w, wv_b, wo_w, wo_b)
    res = run_bass_kernel_spmd(
        nc, in_maps, core_ids=list(range(NCORES)), trace=TRACE)
    LAST_RESULTS[0] = res

    const = (wo_w @ wv_b + wo_b).astype(np.float32)  # V bias folded through softmax
    out = np.zeros((B, S, D), np.float32)
    for c in range(NCORES):
        out[c // 4] += res.results[c]["partial"].reshape(S, D).astype(np.float32)
    out += const[None, None, :]
    return out


# revision 24
# speedup vs baseline: 1.2462x; 1.2462x over previous
"""Multi-head attention (QKV proj + RoPE + SDPA + out proj) on 8 TRN2 NeuronCores.

Sharding: batch x head-group. Core c handles batch c//4 and heads
4*(c%4) .. 4*(c%4)+3 (4 of 16 heads, 256 of 1024 feature dims).

v2 design (all matmuls bf16 except the RoPE swap, N=1024 moving):
  - phase order: K-proj(all) -> Q-proj(qcp0) -> V-proj(all) -> attn(qcp0)
    -> Q-proj(qcp1) -> outproj(qcp0) -> attn(qcp1) -> outproj(qcp1).
    Q-proj(qcp1) fills the PE gap while qcp0's softmax denominators
    normalize; outproj(qcp0) runs while qcp1's attention would stall.
  - projections: x (bf16) resident per-dc tiles, weights bf16; PSUM f32;
    bias via ACT identity into f32 qsb; RoPE swap via pair-swap matmul
    (f32r, N=512 halves); rope mults on DVE in f32; single rounding into
    bf16 qrot/krot.
  - scores TRANSPOSED s[k, q] per head at N=1024; exp on ACT (scale 1/8
    folded, no max subtraction: |s|/8 < ~29, safe in f32/bf16 range).
    Optionally a subset of key-tiles' exps run on DVE as a Schraudolph
    int16/bf16 bit-trick (softmax normalization cancels its constant
    bias; only the +-3% mantissa sawtooth remains) to relieve the ACT
    engine, which is otherwise the attention-phase bottleneck.
  - attn@V with a ones-row per head (M=65): row 64 accumulates the
    softmax denominator free; normalize via reciprocal + partition
    broadcast (gpsimd for overlapped head-pairs, ACT+PE rank-1 for the
    critical tails); ynorm stored bf16.
  - out projection row-parallel, bf16 weights, bf16 partial out; host
    sums partials per batch in f32 and adds wo_b + wo_w @ wv_b (V bias
    commutes through softmax).
"""

import numpy as np
import ml_dtypes

import concourse.bass as bass
import concourse.mybir as mybir
import concourse.tile as tile
from concourse import bacc
import concourse.bass_utils as _bu
from concourse.bass_utils import run_bass_kernel_spmd

F32 = mybir.dt.float32
F32R = mybir.dt.float32r
BF16 = mybir.dt.bfloat16
I16 = mybir.dt.int16
AF = mybir.ActivationFunctionType
OP = mybir.AluOpType

B, S, D = 2, 2048, 1024
NH, HD = 16, 64
NCORES = 8
HPC = 4          # heads per core
DL = HPC * HD    # 256 local dims per core

TRACE = False
LAST_RESULTS = [None]
# key-tiles (of 16) whose exp runs on DVE via the corrected Schraudolph
# bit trick (i=1 head only, to balance ACT vs DVE load)
DVE_EXP_KTS = (3, 7, 11, 15)
STEADY_WARM = 0      # extra filler matmuls per kt in the attention loop
SCHRAUD_A = 0.125 * np.log2(np.e) * 128.0   # score -> int16 exponent scale
SCHRAUD_B = 16256.0                          # 127 * 128
SCHRAUD_D = 135.367   # 128*log2(1 + sqrt2*2^-0.5): centers w0+sqrt2*w1 at 1x
# minimax-ish quadratic for 2^f/(1+f), f = (I & 127)/128
SC_A2, SC_A1, SC_A0 = 0.22697911, -0.21647824, 0.99383134


def _build_module():
    nc = bacc.Bacc("TRN2", target_bir_lowering=False, debug=False)

    xt_d = nc.dram_tensor("xt", [8, 128, S], BF16, kind="ExternalInput")
    wqt_d = nc.dram_tensor("wqt", [128, 8, DL], BF16, kind="ExternalInput")
    wkt_d = nc.dram_tensor("wkt", [128, 8, DL], BF16, kind="ExternalInput")
    wvt_d = nc.dram_tensor("wvt", [128, 8, DL], BF16, kind="ExternalInput")
    wot_d = nc.dram_tensor("wot", [128, 2, D], BF16, kind="ExternalInput")
    qb_d = nc.dram_tensor("qb2", [128, 2], F32, kind="ExternalInput")
    kb_d = nc.dram_tensor("kb2", [128, 2], F32, kind="ExternalInput")
    f0_d = nc.dram_tensor("f0", [128, S], F32, kind="ExternalInput")
    f1_d = nc.dram_tensor("f1", [128, S], F32, kind="ExternalInput")
    psw_d = nc.dram_tensor("pswap", [128, 128], F32R, kind="ExternalInput")
    o164_d = nc.dram_tensor("ones164", [1, 64], F32R, kind="ExternalInput")
    out_d = nc.dram_tensor("partial", [16, 128, D], BF16, kind="ExternalOutput")

    def act_reciprocal(out, in_):
        # ACT-engine reciprocal via direct emission (measured 1.2e-5 max rel)
        eng = nc.scalar
        ins_ = [eng.lower_ap(in_),
                mybir.ImmediateValue(dtype=F32, value=0.0),
                mybir.ImmediateValue(dtype=F32, value=1.0),
                mybir.ImmediateValue(dtype=F32, value=0.0)]
        eng.add_instruction(mybir.InstActivation(
            name=nc.get_next_instruction_name(),
            func=mybir.ActivationFunctionType.Reciprocal,
            ins=ins_, outs=[eng.lower_ap(out)]))

    with tile.TileContext(nc) as tc:
        with (
            tc.tile_pool(name="wts", bufs=1) as wpool,
            tc.tile_pool(name="persist", bufs=1) as ppool,
        ):
            # ---- weights / constants / x (resident), DMA'd in use order ----
            wkt = wpool.tile([128, 8, DL], BF16, tag="wkt")
            nc.sync.dma_start(out=wkt[:], in_=wkt_d.ap())
            xt_sb = []
            for dc in range(8):
                t = wpool.tile([128, S], BF16, tag=f"xt{dc}")
                nc.sync.dma_start(out=t[:], in_=xt_d.ap()[dc])
                xt_sb.append(t)
            kb = wpool.tile([128, 2], F32, tag="kb")
            nc.sync.dma_start(out=kb[:], in_=kb_d.ap())
            f0 = wpool.tile([128, S], F32, tag="f0")
            nc.sync.dma_start(out=f0[:], in_=f0_d.ap())
            f1 = wpool.tile([128, S], F32, tag="f1")
            nc.sync.dma_start(out=f1[:], in_=f1_d.ap())
            psw = wpool.tile([128, 128], F32R, tag="pswap")
            nc.sync.dma_start(out=psw[:], in_=psw_d.ap())
            wqt = wpool.tile([128, 8, DL], BF16, tag="wqt")
            nc.sync.dma_start(out=wqt[:], in_=wqt_d.ap())
            qb = wpool.tile([128, 2], F32, tag="qb")
            nc.sync.dma_start(out=qb[:], in_=qb_d.ap())
            wvt = wpool.tile([128, 8, DL], BF16, tag="wvt")
            nc.sync.dma_start(out=wvt[:], in_=wvt_d.ap())
            wot = wpool.tile([128, 2, D], BF16, tag="wot")
            nc.sync.dma_start(out=wot[:], in_=wot_d.ap())
            o164 = wpool.tile([1, 64], F32R, tag="o164")
            nc.sync.dma_start(out=o164[:], in_=o164_d.ap())

            # Schraudolph additive constants (exact in f32)
            scb = scb2 = None
            if DVE_EXP_KTS:
                scb = wpool.tile([128, 1024], F32, tag="scb")
                nc.vector.memset(scb[:], SCHRAUD_B - SCHRAUD_D)
                scb2 = wpool.tile([128, 1024], F32, tag="scb2")
                nc.vector.memset(scb2[:], SCHRAUD_B - 64.0 - SCHRAUD_D)

            # persistent activations
            qrot = [ppool.tile([128, S], BF16, tag=f"qrot{pt}", name=f"qrot{pt}")
                     for pt in range(2)]
            krot = [ppool.tile([128, S], BF16, tag=f"krot{pt}", name=f"krot{pt}")
                     for pt in range(2)]
            ynorm = [ppool.tile([128, S], BF16, tag=f"ynorm{pt}", name=f"ynorm{pt}")
                     for pt in range(2)]
            vsb = [ppool.tile([128, 4, 65], BF16, tag=f"v{kt}", name=f"vsb{kt}")
                   for kt in range(16)]

            # preload the ACT exp table during the DMA lead-in
            warmact = wpool.tile([1, 1], F32, tag="warmact")
            nc.vector.memset(warmact[:], 0.0)
            nc.scalar.activation(warmact[:], warmact[:], AF.Exp, scale=1.0)

            def emit_proj(wt, bvec, rot, qc, pt, pp, swp, tp):
                """Project + RoPE one [1024-token x 128-dim] block."""
                tsl = slice(qc * 1024, (qc + 1) * 1024)
                qp = pp.tile([128, 1024], F32, tag="qp")
                for qh in range(2):
                    hsl = slice(qc * 1024 + qh * 512, qc * 1024 + (qh + 1) * 512)
                    for dc in range(8):
                        nc.tensor.matmul(
                            qp[:, qh * 512:(qh + 1) * 512],
                            wt[:, dc, pt * 128:(pt + 1) * 128],
                            xt_sb[dc][:, hsl], start=(dc == 0), stop=(dc == 7))
                qsb = tp.tile([128, 1024], F32R, tag="qsb")
                nc.scalar.activation(qsb[:], qp[:], AF.Identity,
                                     bias=bvec[:, pt:pt + 1], scale=1.0)
                sw = swp.tile([128, 1024], F32, tag="sw")
                for qh in range(2):
                    nc.tensor.matmul(sw[:, qh * 512:(qh + 1) * 512], psw[:],
                                     qsb[:, qh * 512:(qh + 1) * 512],
                                     start=True, stop=True)
                t0 = tp.tile([128, 1024], F32, tag="t0")
                nc.vector.tensor_tensor(t0[:], qsb[:], f0[:, tsl], OP.mult)
                t1 = tp.tile([128, 1024], F32, tag="t1")
                nc.vector.tensor_tensor(t1[:], sw[:], f1[:, tsl], OP.mult)
                nc.vector.tensor_tensor(rot[pt][:, tsl], t0[:], t1[:], OP.add)

            # ---- phase 1: K proj (all tokens), Q proj (qcp0) ----
            with (
                tc.tile_pool(name="ptmp", bufs=1) as tp,
                tc.tile_pool(name="pp", bufs=2, space="PSUM") as pp,
                tc.tile_pool(name="swp", bufs=2, space="PSUM") as swp,
            ):
                for qc in range(2):
                    for pt in range(2):
                        emit_proj(wkt, kb, krot, qc, pt, pp, swp, tp)
                for pt in range(2):
                    emit_proj(wqt, qb, qrot, 0, pt, pp, swp, tp)

            # ---- phase 2: V proj (token-major, all 16 key tiles) ----
            with tc.tile_pool(name="vps", bufs=2, space="PSUM") as vps:
                for kt in range(16):
                    vp = vps.tile([128, 256], F32, tag="vp")
                    for dc in range(8):
                        nc.tensor.matmul(
                            vp[:], xt_sb[dc][:, kt * 128:(kt + 1) * 128],
                            wvt[:, dc, :], start=(dc == 0), stop=(dc == 7))
                    nc.vector.tensor_copy(
                        vsb[kt][:, :, 0:64],
                        vp[:].rearrange("p (h c) -> p h c", c=64))
                    nc.vector.memset(vsb[kt][:, :, 64:65], 1.0)

            def warm_attn(aps, n):
                # dep-free fillers (krot only, stable) that keep the PE
                # clock-gate warm; target psum is overwritten right after
                for w in range(n):
                    wt_ = aps.tile([128, 1024], F32, tag="s0", name="warm")
                    nc.tensor.matmul(
                        wt_[:, 0:512], krot[0][0:64, 0:128],
                        krot[0][0:64, 0:512], start=True, stop=True)

            def emit_attention(qcp, epool, scpool, ypool, rpool, aps):
                q0 = qcp * 1024
                ysbs, recs = [], []
                for hp in range(2):
                    pt = hp
                    warm_attn(aps, 3)
                    yps = [aps.tile([65, 1024], F32, tag=f"y{i}", name=f"yps{i}")
                           for i in range(2)]

                    def emit_scores_exp(kt, nwarm=0):
                        exs = []
                        for i in range(2):
                            if i == 0:
                                warm_attn(aps, nwarm)
                            sp = aps.tile([128, 1024], F32, tag=f"s{i}")
                            po = 64 * i
                            for qh in range(2):
                                nc.tensor.matmul(
                                    sp[:, qh * 512:(qh + 1) * 512],
                                    krot[pt][po:po + 64, kt * 128:(kt + 1) * 128],
                                    qrot[pt][po:po + 64,
                                             q0 + qh * 512:q0 + (qh + 1) * 512],
                                    start=True, stop=True)
                            if kt in DVE_EXP_KTS and i == 1:
                                # two-point Schraudolph exp on DVE:
                                # w0 = bitcast(round(A*s + B)) as bf16,
                                # w1 = bitcast(round(A*s + B - 64)); the
                                # half-period offset cancels most of the
                                # linear-mantissa sawtooth:
                                # w = w0 + sqrt(2)*w1 (constant factor
                                # cancels in softmax normalization)
                                exi = scpool.tile([128, 1024], I16, tag=f"ei{i}")
                                nc.vector.scalar_tensor_tensor(
                                    exi[:], sp[:], float(SCHRAUD_A), scb[:],
                                    OP.mult, OP.add)
                                exj = scpool.tile([128, 1024], I16, tag=f"ej{i}")
                                nc.vector.scalar_tensor_tensor(
                                    exj[:], sp[:], float(SCHRAUD_A), scb2[:],
                                    OP.mult, OP.add)
                                exf = epool.tile([128, 1024], BF16, tag=f"ex{i}")
                                nc.vector.scalar_tensor_tensor(
                                    exf[:], exj[:].bitcast(BF16), 1.41421356,
                                    exi[:].bitcast(BF16), OP.mult, OP.add)
                                exs.append(exf)
                            else:
                                ext = epool.tile([128, 1024], BF16, tag=f"e{i}")
                                nc.scalar.activation(ext[:], sp[:], AF.Exp,
                                                     scale=0.125)
                                exs.append(ext)
                        return exs

                    def emit_attnv(kt, exs):
                        for i in range(2):
                            h = 2 * hp + i
                            for qh in range(2):
                                nc.tensor.matmul(
                                    yps[i][:, qh * 512:(qh + 1) * 512],
                                    vsb[kt][:, h, :],
                                    exs[i][:, qh * 512:(qh + 1) * 512],
                                    start=(kt == 0), stop=(kt == 15))

                    prev = emit_scores_exp(0, 3)
                    for kt in range(1, 16):
                        exs = emit_scores_exp(kt, 2 if kt < 3 else STEADY_WARM)
                        emit_attnv(kt - 1, prev)
                        prev = exs
                    emit_attnv(15, prev)

                    # normalize. reciprocal = Exp(-Log(d)) on ACT: both
                    # functions share the loaded table set, so no
                    # ACT_TABLE_LOAD swaps mid-phase. hp0's broadcast goes
                    # via gpsimd (overlaps hp1 attention); hp1 uses the
                    # rank-1 PE broadcast for the shorter critical tail.
                    for i in range(2):
                        ysb = ypool.tile([65, 1024], F32, tag=f"ysb{i}")
                        nc.vector.tensor_copy(ysb[:], yps[i][:])
                        lnd = rpool.tile([1, 1024], F32, tag=f"ln{i}")
                        nc.scalar.activation(lnd[:], ysb[64:65, :], AF.Ln,
                                             scale=1.0)
                        rec = rpool.tile([1, 1024], F32R, tag=f"rec{i}")
                        nc.scalar.activation(rec[:], lnd[:], AF.Exp,
                                             scale=-1.0)
                        ysbs.append(ysb)
                        recs.append(rec)
                    for i in range(2):
                        po = 64 * i
                        ysb, rec = ysbs[2 * hp + i], recs[2 * hp + i]
                        if hp == 1:
                            nb = aps.tile([64, 1024], F32, tag=f"y{i}")
                            for qh in range(2):
                                nc.tensor.matmul(
                                    nb[:, qh * 512:(qh + 1) * 512], o164[:],
                                    rec[:, qh * 512:(qh + 1) * 512],
                                    start=True, stop=True)
                            nc.vector.tensor_tensor(
                                ynorm[pt][po:po + 64, q0:q0 + 1024],
                                ysb[0:64, :], nb[:], OP.mult)
                        else:
                            rb = rpool.tile([64, 1024], F32, tag=f"rb{i}")
                            nc.gpsimd.partition_broadcast(
                                rb[:], rec[:].bitcast(F32), channels=64)
                            nc.vector.tensor_tensor(
                                ynorm[pt][po:po + 64, q0:q0 + 1024],
                                ysb[0:64, :], rb[:], OP.mult)

            def emit_outproj(qcp, opp, obuf):
                for w in range(6):
                    wt_ = opp.tile([128, 1024], F32, tag="op0", name="warmo")
                    nc.tensor.matmul(
                        wt_[:, 0:512], krot[0][0:64, 0:128],
                        krot[0][0:64, 0:512], start=True, stop=True)
                for tt in range(8):
                    tg = qcp * 8 + tt
                    op = opp.tile([128, 1024], F32, tag=f"op{tt % 2}")
                    for oc in range(2):
                        for pt2 in range(2):
                            nc.tensor.matmul(
                                op[:, oc * 512:(oc + 1) * 512],
                                ynorm[pt2][:, tg * 128:(tg + 1) * 128],
                                wot[:, pt2, oc * 512:(oc + 1) * 512],
                                start=(pt2 == 0), stop=(pt2 == 1))
                    osb = obuf.tile([128, 1024], BF16, tag=f"osb{tt % 2}")
                    nc.vector.tensor_copy(osb[:], op[:])
                    nc.sync.dma_start(out=out_d.ap()[tg], in_=osb[:])

            with (
                tc.tile_pool(name="exp", bufs=2) as epool,
                tc.tile_pool(name="sct", bufs=1) as scpool,
                tc.tile_pool(name="ysb", bufs=1) as ypool,
                tc.tile_pool(name="rp", bufs=1) as rpool,
                tc.tile_pool(name="obuf", bufs=2) as obuf,
            ):
                with tc.tile_pool(name="aps0", bufs=1, space="PSUM") as aps:
                    emit_attention(0, epool, scpool, ypool, rpool, aps)
                with (
                    tc.tile_pool(name="ptmp1", bufs=1) as tp,
                    tc.tile_pool(name="pp1", bufs=2, space="PSUM") as pp,
                    tc.tile_pool(name="swp1", bufs=2, space="PSUM") as swp,
                ):
                    for pt in range(2):
                        emit_proj(wqt, qb, qrot, 1, pt, pp, swp, tp)
                with tc.tile_pool(name="opp0", bufs=2, space="PSUM") as opp:
                    emit_outproj(0, opp, obuf)
                with tc.tile_pool(name="aps1", bufs=1, space="PSUM") as aps:
                    emit_attention(1, epool, scpool, ypool, rpool, aps)
                with tc.tile_pool(name="opp1", bufs=2, space="PSUM") as opp:
                    emit_outproj(1, opp, obuf)

    nc.compile()
    return nc


_NC = None


def _get_module():
    global _NC
    if _NC is None:
        _NC = _build_module()
    return _NC


def _prep_in_maps(q, freqs_cis, wq_w, wq_b, wk_w, wk_b, wv_w, wv_b, wo_w, wo_b):
    BF = ml_dtypes.bfloat16
    # F0/F1 [128, S] (identical layout for every head pair on 128 partitions)
    i_of_p = (np.arange(128) % HD) // 2
    sign = np.where(np.arange(128) % 2 == 0, -1.0, 1.0).astype(np.float32)
    f0 = freqs_cis[:, i_of_p, 0].T.copy()                 # [128, S]
    f1 = (freqs_cis[:, i_of_p, 1].T * sign[:, None]).copy()
    pswap = np.zeros((128, 128), np.float32)
    idx = np.arange(128)
    pswap[idx ^ 1, idx] = 1.0
    ones164 = np.ones((1, 64), np.float32)

    def warr(w):  # [256, out-rows of W^T] -> [128, 8, 256] bf16
        return np.ascontiguousarray(
            w.T.reshape(8, 128, DL).transpose(1, 0, 2)).astype(BF)

    in_maps = []
    for c in range(NCORES):
        b, hg = c // 4, c % 4
        sl = slice(hg * DL, (hg + 1) * DL)
        in_maps.append({
            "xt": np.ascontiguousarray(q[b].T.reshape(8, 128, S)).astype(BF),
            "wqt": warr(wq_w[sl]),
            "wkt": warr(wk_w[sl]),
            "wvt": warr(wv_w[sl]),
            "wot": np.ascontiguousarray(
                wo_w[:, sl].T.reshape(2, 128, D).transpose(1, 0, 2)).astype(BF),
            "qb2": np.ascontiguousarray(wq_b[sl].reshape(2, 128).T),
            "kb2": np.ascontiguousarray(wk_b[sl].reshape(2, 128).T),
            "f0": f0,
            "f1": f1,
            "pswap": pswap,
            "ones164": ones164,
        })
    return in_maps


def kernel(q, freqs_cis, wq_w, wq_b, wk_w, wk_b, wv_w, wv_b, wo_w, wo_b):
    q = np.asarray(q, np.float32)
    freqs_cis = np.asarray(freqs_cis, np.float32)
    wq_w = np.asarray(wq_w, np.float32)
    wq_b = np.asarray(wq_b, np.float32)
    wk_w = np.asarray(wk_w, np.float32)
    wk_b = np.asarray(wk_b, np.float32)
    wv_w = np.asarray(wv_w, np.float32)
    wv_b = np.asarray(wv_b, np.float32)
    wo_w = np.asarray(wo_w, np.float32)
    wo_b = np.asarray(wo_b, np.float32)

    nc = _get_module()
    in_maps = _prep_in_maps(q, freqs_cis, wq_w, wq_b, wk_w, wk_b,
                            wv_w, wv_b, wo_w, wo_b)
    res = run_bass_kernel_spmd(
        nc, in_maps, core_ids=list(range(NCORES)), trace=TRACE)
    LAST_RESULTS[0] = res

    const = (wo_w @ wv_b + wo_b).astype(np.float32)  # V bias folded through softmax
    out = np.zeros((B, S, D), np.float32)
    for c in range(NCORES):
        out[c // 4] += res.results[c]["partial"].reshape(S, D).astype(np.float32)
    out += const[None, None, :]
    return out


# revision 26
# speedup vs baseline: 1.3322x; 1.0690x over previous
"""Multi-head attention (QKV proj + RoPE + SDPA + out proj) on 8 TRN2 NeuronCores.

Sharding: batch x head-group. Core c handles batch c//4 and heads
4*(c%4) .. 4*(c%4)+3 (4 of 16 heads, 256 of 1024 feature dims).

v2 design (all matmuls bf16 except the RoPE swap, N=1024 moving):
  - phase order: K-proj(all) -> Q-proj(qcp0) -> V-proj(all) -> attn(qcp0)
    -> Q-proj(qcp1) -> outproj(qcp0) -> attn(qcp1) -> outproj(qcp1).
    Q-proj(qcp1) fills the PE gap while qcp0's softmax denominators
    normalize; outproj(qcp0) runs while qcp1's attention would stall.
  - projections: x (bf16) resident per-dc tiles, weights bf16; PSUM f32;
    bias via ACT identity into f32 qsb; RoPE swap via pair-swap matmul
    (f32r, N=512 halves); rope mults on DVE in f32; single rounding into
    bf16 qrot/krot.
  - scores TRANSPOSED s[k, q] per head at N=1024; exp on ACT (scale 1/8
    folded, no max subtraction: |s|/8 < ~29, safe in f32/bf16 range).
    Optionally a subset of key-tiles' exps run on DVE as a Schraudolph
    int16/bf16 bit-trick (softmax normalization cancels its constant
    bias; only the +-3% mantissa sawtooth remains) to relieve the ACT
    engine, which is otherwise the attention-phase bottleneck.
  - attn@V with a ones-row per head (M=65): row 64 accumulates the
    softmax denominator free; normalize via reciprocal + partition
    broadcast (gpsimd for overlapped head-pairs, ACT+PE rank-1 for the
    critical tails); ynorm stored bf16.
  - out projection row-parallel, bf16 weights, bf16 partial out; host
    sums partials per batch in f32 and adds wo_b + wo_w @ wv_b (V bias
    commutes through softmax).
"""

import numpy as np
import ml_dtypes

import concourse.bass as bass
import concourse.mybir as mybir
import concourse.tile as tile
from concourse import bacc
import concourse.bass_utils as _bu
from concourse.bass_utils import run_bass_kernel_spmd

F32 = mybir.dt.float32
F32R = mybir.dt.float32r
BF16 = mybir.dt.bfloat16
I16 = mybir.dt.int16
AF = mybir.ActivationFunctionType
OP = mybir.AluOpType

B, S, D = 2, 2048, 1024
NH, HD = 16, 64
NCORES = 8
HPC = 4          # heads per core
DL = HPC * HD    # 256 local dims per core

TRACE = False
LAST_RESULTS = [None]
# key-tiles (of 16) whose exp runs on DVE via the corrected Schraudolph
# bit trick (i=1 head only, to balance ACT vs DVE load)
DVE_EXP_KTS = (2, 5, 8, 11)
STEADY_WARM = 0      # extra filler matmuls per kt in the attention loop
SCHRAUD_A = 0.125 * np.log2(np.e) * 128.0   # score -> int16 exponent scale
SCHRAUD_B = 16256.0                          # 127 * 128
SCHRAUD_D = 135.367   # 128*log2(1 + sqrt2*2^-0.5): centers w0+sqrt2*w1 at 1x
# minimax-ish quadratic for 2^f/(1+f), f = (I & 127)/128
SC_A2, SC_A1, SC_A0 = 0.22697911, -0.21647824, 0.99383134


def _build_module():
    nc = bacc.Bacc("TRN2", target_bir_lowering=False, debug=False)

    xt_d = nc.dram_tensor("xt", [8, 128, S], BF16, kind="ExternalInput")
    wqt_d = nc.dram_tensor("wqt", [128, 8, DL], BF16, kind="ExternalInput")
    wkt_d = nc.dram_tensor("wkt", [128, 8, DL], BF16, kind="ExternalInput")
    wvt_d = nc.dram_tensor("wvt", [128, 8, DL], BF16, kind="ExternalInput")
    wot_d = nc.dram_tensor("wot", [128, 2, D], BF16, kind="ExternalInput")
    qb_d = nc.dram_tensor("qb2", [128, 2], F32, kind="ExternalInput")
    kb_d = nc.dram_tensor("kb2", [128, 2], F32, kind="ExternalInput")
    f0_d = nc.dram_tensor("f0", [128, S], F32, kind="ExternalInput")
    f1_d = nc.dram_tensor("f1", [128, S], F32, kind="ExternalInput")
    psw_d = nc.dram_tensor("pswap", [128, 128], F32R, kind="ExternalInput")
    o164_d = nc.dram_tensor("ones164", [1, 64], F32R, kind="ExternalInput")
    out_d = nc.dram_tensor("partial", [16, 128, D], BF16, kind="ExternalOutput")

    def act_reciprocal(out, in_):
        # ACT-engine reciprocal via direct emission (measured 1.2e-5 max rel)
        eng = nc.scalar
        ins_ = [eng.lower_ap(in_),
                mybir.ImmediateValue(dtype=F32, value=0.0),
                mybir.ImmediateValue(dtype=F32, value=1.0),
                mybir.ImmediateValue(dtype=F32, value=0.0)]
        eng.add_instruction(mybir.InstActivation(
            name=nc.get_next_instruction_name(),
            func=mybir.ActivationFunctionType.Reciprocal,
            ins=ins_, outs=[eng.lower_ap(out)]))

    with tile.TileContext(nc) as tc:
        with (
            tc.tile_pool(name="wts", bufs=1) as wpool,
            tc.tile_pool(name="persist", bufs=1) as ppool,
        ):
            # ---- weights / constants / x (resident), DMA'd in use order ----
            wkt = wpool.tile([128, 8, DL], BF16, tag="wkt")
            nc.sync.dma_start(out=wkt[:], in_=wkt_d.ap())
            xt_sb = []
            for dc in range(8):
                t = wpool.tile([128, S], BF16, tag=f"xt{dc}")
                nc.sync.dma_start(out=t[:], in_=xt_d.ap()[dc])
                xt_sb.append(t)
            kb = wpool.tile([128, 2], F32, tag="kb")
            nc.sync.dma_start(out=kb[:], in_=kb_d.ap())
            f0 = wpool.tile([128, S], F32, tag="f0")
            nc.sync.dma_start(out=f0[:], in_=f0_d.ap())
            f1 = wpool.tile([128, S], F32, tag="f1")
            nc.sync.dma_start(out=f1[:], in_=f1_d.ap())
            psw = wpool.tile([128, 128], F32R, tag="pswap")
            nc.sync.dma_start(out=psw[:], in_=psw_d.ap())
            wqt = wpool.tile([128, 8, DL], BF16, tag="wqt")
            nc.sync.dma_start(out=wqt[:], in_=wqt_d.ap())
            qb = wpool.tile([128, 2], F32, tag="qb")
            nc.sync.dma_start(out=qb[:], in_=qb_d.ap())
            wvt = wpool.tile([128, 8, DL], BF16, tag="wvt")
            nc.sync.dma_start(out=wvt[:], in_=wvt_d.ap())
            wot = wpool.tile([128, 2, D], BF16, tag="wot")
            nc.sync.dma_start(out=wot[:], in_=wot_d.ap())
            o164 = wpool.tile([1, 64], F32R, tag="o164")
            nc.sync.dma_start(out=o164[:], in_=o164_d.ap())

            # Schraudolph additive constants (exact in f32)
            scb = scb2 = None
            if DVE_EXP_KTS:
                scb = wpool.tile([128, 1024], F32, tag="scb")
                nc.vector.memset(scb[:], SCHRAUD_B - SCHRAUD_D)
                scb2 = wpool.tile([128, 1024], F32, tag="scb2")
                nc.vector.memset(scb2[:], SCHRAUD_B - 64.0 - SCHRAUD_D)

            # persistent activations
            qrot = [ppool.tile([128, S], BF16, tag=f"qrot{pt}", name=f"qrot{pt}")
                     for pt in range(2)]
            krot = [ppool.tile([128, S], BF16, tag=f"krot{pt}", name=f"krot{pt}")
                     for pt in range(2)]
            ynorm = [ppool.tile([128, S], BF16, tag=f"ynorm{pt}", name=f"ynorm{pt}")
                     for pt in range(2)]
            vsb = [ppool.tile([128, 4, 65], BF16, tag=f"v{kt}", name=f"vsb{kt}")
                   for kt in range(16)]

            # preload the ACT exp table during the DMA lead-in
            warmact = wpool.tile([1, 1], F32, tag="warmact")
            nc.vector.memset(warmact[:], 0.0)
            nc.scalar.activation(warmact[:], warmact[:], AF.Exp, scale=1.0)

            def emit_proj(wt, bvec, rot, qc, pt, pp, swp, tp):
                """Project + RoPE one [1024-token x 128-dim] block."""
                tsl = slice(qc * 1024, (qc + 1) * 1024)
                qp = pp.tile([128, 1024], F32, tag="qp")
                for qh in range(2):
                    hsl = slice(qc * 1024 + qh * 512, qc * 1024 + (qh + 1) * 512)
                    for dc in range(8):
                        nc.tensor.matmul(
                            qp[:, qh * 512:(qh + 1) * 512],
                            wt[:, dc, pt * 128:(pt + 1) * 128],
                            xt_sb[dc][:, hsl], start=(dc == 0), stop=(dc == 7))
                qsb = tp.tile([128, 1024], F32R, tag="qsb")
                nc.scalar.activation(qsb[:], qp[:], AF.Identity,
                                     bias=bvec[:, pt:pt + 1], scale=1.0)
                sw = swp.tile([128, 1024], F32, tag="sw")
                for qh in range(2):
                    nc.tensor.matmul(sw[:, qh * 512:(qh + 1) * 512], psw[:],
                                     qsb[:, qh * 512:(qh + 1) * 512],
                                     start=True, stop=True)
                t0 = tp.tile([128, 1024], F32, tag="t0")
                nc.vector.tensor_tensor(t0[:], qsb[:], f0[:, tsl], OP.mult)
                t1 = tp.tile([128, 1024], F32, tag="t1")
                nc.vector.tensor_tensor(t1[:], sw[:], f1[:, tsl], OP.mult)
                nc.vector.tensor_tensor(rot[pt][:, tsl], t0[:], t1[:], OP.add)

            # ---- phase 1: K proj (all tokens), Q proj (qcp0) ----
            with (
                tc.tile_pool(name="ptmp", bufs=1) as tp,
                tc.tile_pool(name="pp", bufs=2, space="PSUM") as pp,
                tc.tile_pool(name="swp", bufs=2, space="PSUM") as swp,
            ):
                for qc in range(2):
                    for pt in range(2):
                        emit_proj(wkt, kb, krot, qc, pt, pp, swp, tp)
                for pt in range(2):
                    emit_proj(wqt, qb, qrot, 0, pt, pp, swp, tp)

            # ---- phase 2: V proj (token-major, all 16 key tiles) ----
            with tc.tile_pool(name="vps", bufs=2, space="PSUM") as vps:
                for kt in range(16):
                    vp = vps.tile([128, 256], F32, tag="vp")
                    for dc in range(8):
                        nc.tensor.matmul(
                            vp[:], xt_sb[dc][:, kt * 128:(kt + 1) * 128],
                            wvt[:, dc, :], start=(dc == 0), stop=(dc == 7))
                    nc.vector.tensor_copy(
                        vsb[kt][:, :, 0:64],
                        vp[:].rearrange("p (h c) -> p h c", c=64))
                    nc.vector.memset(vsb[kt][:, :, 64:65], 1.0)

            def warm_attn(aps, n):
                # dep-free fillers (krot only, stable) that keep the PE
                # clock-gate warm; target psum is overwritten right after
                for w in range(n):
                    wt_ = aps.tile([128, 1024], F32, tag="s0", name="warm")
                    nc.tensor.matmul(
                        wt_[:, 0:512], krot[0][0:64, 0:128],
                        krot[0][0:64, 0:512], start=True, stop=True)

            def emit_attention(qcp, epool, scpool, ypool, rpool, aps):
                """Attention for one 1024-query chunk. attn@V trails the
                scores/exp stage by TWO key-tiles so even the 3-op DVE exp
                (3.7us) lands before its consumer. Normalize is emitted
                engine-only (DVE/ACT/gpsimd) for hp0; hp1's PE part (rank-1
                broadcast + mult) is returned as deferred jobs so dep-free
                matmuls of the next phase can be queued first (the PE queue
                is in-order)."""
                q0 = qcp * 1024
                jobs = []
                for hp in range(2):
                    pt = hp
                    warm_attn(aps, 3)
                    yps = [aps.tile([65, 1024], F32, tag=f"y{i}", name=f"yps{i}")
                           for i in range(2)]

                    def emit_scores_exp(kt, nwarm=0):
                        exs = []
                        for i in range(2):
                            if i == 0:
                                warm_attn(aps, nwarm)
                            sp = aps.tile([128, 1024], F32, tag=f"s{i}")
                            po = 64 * i
                            for qh in range(2):
                                nc.tensor.matmul(
                                    sp[:, qh * 512:(qh + 1) * 512],
                                    krot[pt][po:po + 64, kt * 128:(kt + 1) * 128],
                                    qrot[pt][po:po + 64,
                                             q0 + qh * 512:q0 + (qh + 1) * 512],
                                    start=True, stop=True)
                            if kt in DVE_EXP_KTS and i == 1:
                                # two-point Schraudolph exp on DVE:
                                # w0 = bitcast(round(A*s + B0)) as bf16,
                                # w1 = bitcast(round(A*s + B0 - 64)); the
                                # half-period offset cancels most of the
                                # linear-mantissa sawtooth; B0 is shifted
                                # by D so w0 + sqrt2*w1 is centered at 1x
                                # (must match the ACT tiles' scale!)
                                exi = scpool.tile([128, 1024], I16, tag=f"ei{i}")
                                nc.vector.scalar_tensor_tensor(
                                    exi[:], sp[:], float(SCHRAUD_A), scb[:],
                                    OP.mult, OP.add)
                                exj = scpool.tile([128, 1024], I16, tag=f"ej{i}")
                                nc.vector.scalar_tensor_tensor(
                                    exj[:], sp[:], float(SCHRAUD_A), scb2[:],
                                    OP.mult, OP.add)
                                exf = epool.tile([128, 1024], BF16, tag=f"ex{i}")
                                nc.vector.scalar_tensor_tensor(
                                    exf[:], exj[:].bitcast(BF16), 1.41421356,
                                    exi[:].bitcast(BF16), OP.mult, OP.add)
                                exs.append(exf)
                            else:
                                ext = epool.tile([128, 1024], BF16, tag=f"e{i}")
                                nc.scalar.activation(ext[:], sp[:], AF.Exp,
                                                     scale=0.125)
                                exs.append(ext)
                        return exs

                    def emit_attnv(kt, exs):
                        for i in range(2):
                            h = 2 * hp + i
                            for qh in range(2):
                                nc.tensor.matmul(
                                    yps[i][:, qh * 512:(qh + 1) * 512],
                                    vsb[kt][:, h, :],
                                    exs[i][:, qh * 512:(qh + 1) * 512],
                                    start=(kt == 0), stop=(kt == 15))

                    prevs = []
                    for kt in range(16):
                        nwarm = 3 if kt == 0 else (2 if kt < 3 else STEADY_WARM)
                        prevs.append(emit_scores_exp(kt, nwarm))
                        if kt >= 2:
                            emit_attnv(kt - 2, prevs[kt - 2])
                    emit_attnv(14, prevs[14])
                    emit_attnv(15, prevs[15])

                    # normalize: reciprocal = Exp(-Ln(d)) on ACT (same table
                    # set as the softmax exps -> no ACT_TABLE_LOAD swaps)
                    for i in range(2):
                        po = 64 * i
                        ysb = ypool.tile([65, 1024], F32, tag=f"ysb{hp}{i}")
                        nc.vector.tensor_copy(ysb[:], yps[i][:])
                        lnd = rpool.tile([1, 1024], F32, tag=f"ln{hp}{i}")
                        nc.scalar.activation(lnd[:], ysb[64:65, :], AF.Ln,
                                             scale=1.0)
                        rec = rpool.tile([1, 1024], F32R, tag=f"rec{hp}{i}")
                        nc.scalar.activation(rec[:], lnd[:], AF.Exp,
                                             scale=-1.0)
                        if hp == 0:
                            # engine-only path, overlaps hp1's attention
                            rb = rpool.tile([64, 1024], F32, tag=f"rb{i}")
                            nc.gpsimd.partition_broadcast(
                                rb[:], rec[:].bitcast(F32), channels=64)
                            nc.vector.tensor_tensor(
                                ynorm[pt][po:po + 64, q0:q0 + 1024],
                                ysb[0:64, :], rb[:], OP.mult)
                        else:
                            jobs.append((pt, po, ysb, rec))
                return jobs

            def emit_norm_pe(qcp, jobs, nbpool):
                # deferred rank-1 PE broadcast + DVE mult for hp1's heads
                q0 = qcp * 1024
                for idx, (pt, po, ysb, rec) in enumerate(jobs):
                    nb = nbpool.tile([64, 1024], F32, tag=f"nb{idx}",
                                     name=f"nb{idx}")
                    for qh in range(2):
                        nc.tensor.matmul(
                            nb[:, qh * 512:(qh + 1) * 512], o164[:],
                            rec[:, qh * 512:(qh + 1) * 512],
                            start=True, stop=True)
                    nc.vector.tensor_tensor(
                        ynorm[pt][po:po + 64, q0:q0 + 1024],
                        ysb[0:64, :], nb[:], OP.mult)

            def emit_outproj(qcp, opp, obuf, nwarm):
                for w in range(nwarm):
                    wt_ = opp.tile([128, 1024], F32, tag="op0", name="warmo")
                    nc.tensor.matmul(
                        wt_[:, 0:512], krot[0][0:64, 0:128],
                        krot[0][0:64, 0:512], start=True, stop=True)

            def emit_outproj_mms(qcp, opp, obuf):
                for tt in range(8):
                    tg = qcp * 8 + tt
                    op = opp.tile([128, 1024], F32, tag=f"op{tt % 2}")
                    for oc in range(2):
                        for pt2 in range(2):
                            nc.tensor.matmul(
                                op[:, oc * 512:(oc + 1) * 512],
                                ynorm[pt2][:, tg * 128:(tg + 1) * 128],
                                wot[:, pt2, oc * 512:(oc + 1) * 512],
                                start=(pt2 == 0), stop=(pt2 == 1))
                    osb = obuf.tile([128, 1024], BF16, tag=f"osb{tt % 2}")
                    nc.vector.tensor_copy(osb[:], op[:])
                    nc.sync.dma_start(out=out_d.ap()[tg], in_=osb[:])

            with (
                tc.tile_pool(name="exp", bufs=3) as epool,
                tc.tile_pool(name="sct", bufs=1) as scpool,
                tc.tile_pool(name="ysb", bufs=1) as ypool,
                tc.tile_pool(name="rp", bufs=1) as rpool,
                tc.tile_pool(name="obuf", bufs=2) as obuf,
            ):
                with tc.tile_pool(name="aps0", bufs=1, space="PSUM") as aps:
                    jobs0 = emit_attention(0, epool, scpool, ypool, rpool, aps)
                with (
                    tc.tile_pool(name="ptmp1", bufs=1) as tp,
                    tc.tile_pool(name="pp1", bufs=1, space="PSUM") as pp,
                    tc.tile_pool(name="swp1", bufs=1, space="PSUM") as swp,
                    tc.tile_pool(name="nbp0", bufs=1, space="PSUM") as nbp,
                ):
                    # Q1-proj matmuls are dep-free: they keep the PE busy
                    # while qcp0's hp1 denominators normalize
                    for pt in range(2):
                        emit_proj(wqt, qb, qrot, 1, pt, pp, swp, tp)
                    emit_norm_pe(0, jobs0, nbp)
                with tc.tile_pool(name="opp0", bufs=2, space="PSUM") as opp:
                    emit_outproj(0, opp, obuf, 2)
                    emit_outproj_mms(0, opp, obuf)
                with tc.tile_pool(name="aps1", bufs=1, space="PSUM") as aps:
                    jobs1 = emit_attention(1, epool, scpool, ypool, rpool, aps)
                with (
                    tc.tile_pool(name="opp1", bufs=1, space="PSUM") as opp,
                    tc.tile_pool(name="nbp1", bufs=1, space="PSUM") as nbp,
                ):
                    emit_outproj(1, opp, obuf, 8)
                    emit_norm_pe(1, jobs1, nbp)
                    emit_outproj_mms(1, opp, obuf)

    nc.compile()
    return nc


_NC = None


def _get_module():
    global _NC
    if _NC is None:
        _NC = _build_module()
    return _NC


def _prep_in_maps(q, freqs_cis, wq_w, wq_b, wk_w, wk_b, wv_w, wv_b, wo_w, wo_b):
    BF = ml_dtypes.bfloat16
    # F0/F1 [128, S] (identical layout for every head pair on 128 partitions)
    i_of_p = (np.arange(128) % HD) // 2
    sign = np.where(np.arange(128) % 2 == 0, -1.0, 1.0).astype(np.float32)
    f0 = freqs_cis[:, i_of_p, 0].T.copy()                 # [128, S]
    f1 = (freqs_cis[:, i_of_p, 1].T * sign[:, None]).copy()
    pswap = np.zeros((128, 128), np.float32)
    idx = np.arange(128)
    pswap[idx ^ 1, idx] = 1.0
    ones164 = np.ones((1, 64), np.float32)

    def warr(w):  # [256, out-rows of W^T] -> [128, 8, 256] bf16
        return np.ascontiguousarray(
            w.T.reshape(8, 128, DL).transpose(1, 0, 2)).astype(BF)

    in_maps = []
    for c in range(NCORES):
        b, hg = c // 4, c % 4
        sl = slice(hg * DL, (hg + 1) * DL)
        in_maps.append({
            "xt": np.ascontiguousarray(q[b].T.reshape(8, 128, S)).astype(BF),
            "wqt": warr(wq_w[sl]),
            "wkt": warr(wk_w[sl]),
            "wvt": warr(wv_w[sl]),
            "wot": np.ascontiguousarray(
                wo_w[:, sl].T.reshape(2, 128, D).transpose(1, 0, 2)).astype(BF),
            "qb2": np.ascontiguousarray(wq_b[sl].reshape(2, 128).T),
            "kb2": np.ascontiguousarray(wk_b[sl].reshape(2, 128).T),
            "f0": f0,
            "f1": f1,
            "pswap": pswap,
            "ones164": ones164,
        })
    return in_maps


def kernel(q, freqs_cis, wq_w, wq_b, wk_w, wk_b, wv_w, wv_b, wo_w, wo_b):
    q = np.asarray(q, np.float32)
    freqs_cis = np.asarray(freqs_cis, np.float32)
    wq_w = np.asarray(wq_w, np.float32)
    wq_b = np.asarray(wq_b, np.float32)
    wk_w = np.asarray(wk_w, np.float32)
    wk_b = np.asarray(wk_b, np.float32)
    wv_w = np.asarray(wv_w, np.float32)
    wv_b = np.asarray(wv_b, np.float32)
    wo_w = np.asarray(wo_w, np.float32)
    wo_b = np.asarray(wo_b, np.float32)

    nc = _get_module()
    in_maps = _prep_in_maps(q, freqs_cis, wq_w, wq_b, wk_w, wk_b,
                            wv_w, wv_b, wo_w, wo_b)
    res = run_bass_kernel_spmd(
        nc, in_maps, core_ids=list(range(NCORES)), trace=TRACE)
    LAST_RESULTS[0] = res

    const = (wo_w @ wv_b + wo_b).astype(np.float32)  # V bias folded through softmax
    out = np.zeros((B, S, D), np.float32)
    for c in range(NCORES):
        out[c // 4] += res.results[c]["partial"].reshape(S, D).astype(np.float32)
    out += const[None, None, :]
    return out
